# revision 24
# baseline (speedup 1.0000x reference)
"""Two-layer Elman RNN (B=64, S=512, EMB=512, HID=1024) on 8 TRN2 NeuronCores.

Layer-pipelined pairs: pair p = (core p, core p+4) handles batch quarter p
(16 rows). Core p runs the layer-1 scan; core p+4 runs the layer-2 scan LAG=2
chunks behind. The pair exchanges RAW h archives (fp16, feature-major) via a
pair-wise AllGather each chunk; each core then computes its own scan input
for the next chunk in a single unified "production" matmul pass
pre = SRC @ Wprod + bprod, where the per-core weight input Wprod is
[Wi1; 0] on layer-1 cores and Wi2 on layer-2 cores, and SRC is selected
per-core by an indirect row-gather from a combined DRAM buffer (layer-1
cores pick their own embedded-token slab, layer-2 cores pick the partner's
h1 chunk). All 8 cores execute an identical SPMD program — only input data
differs.

The scan step is built around the timeline cost structure: each step is ONE
PSUM accumulation group — an identity-stationary matmul injects the produced
input chunk, 64 Wh tile matmuls accumulate on top, and a single [128,128]
Tanh drains the bank into the feature-major h archive. Only two cross-engine
semaphore hops sit on the per-step critical chain; the identity matmul
depends only on chunk-level data so it executes under the previous step's
tanh. All off-scan PE work (embedding transposes, production matmuls) is
emitted in small work items BETWEEN scan steps so it fills the per-step tanh
windows instead of bunching into a serial block at chunk boundaries.
Matmuls in fp16 (fp32 PSUM accumulate), tanh and sigmoid in fp32.

The zero fixed point of h = tanh(W h + 0) makes the warm-up chunks of the
layer-2 cores (whose prologue production weights are zero) end exactly in
the correct initial state h = 0. (Assumes b2 == 0 for the warm-up, which
holds for this model family.)
"""

from contextlib import ExitStack

import numpy as np

import concourse.bass as bass
import concourse.bacc as bacc
import concourse.mybir as mybir
import concourse.tile as tile
from concourse.bass import IndirectOffsetOnAxis
from concourse.bass_utils import run_bass_kernel_spmd
from concourse.masks import make_identity

P = 128
VOCAB, EMB, HID = 50257, 512, 1024
B, S = 64, 512
NCORES = 8
NPAIR = 4
BL = B // NPAIR           # batch rows per pair = 16
KE = EMB // P             # 4
KH = HID // P             # 8
M = HID // P              # 8
CS = 32                   # scan steps per chunk
NCH = S // CS             # 16 chunks
LAG = 2                   # chunks the layer-2 scan trails the layer-1 scan
CCOL = BL * CS            # 512 token-columns per chunk
W = M * BL                # 128 columns per step block (m, b)
CW = CS * W               # 4096 columns per chunk in combined layout
HCW = CW // 2             # half-chunk columns
SBL = S * BL // P         # 64 token-column groups total

CDT = mybir.dt.float16
NP_CDT = np.float16
F32 = mybir.dt.float32
I32 = mybir.dt.int32

_BUILT = {}
REPLICA_GROUPS = [[p, p + NPAIR] for p in range(NPAIR)]


def _tb(ap, b=BL):
    return ap.rearrange("p (t b) -> p t b", b=b)


def _tmb(ap, m):
    """m-th [P, CS, BL] slice of a [P, CW] combined (t,m,b) AP."""
    return ap.rearrange("p (t mb) -> p t mb", mb=W)[:, :, m * BL:(m + 1) * BL]


def build(local_cc=False):
    """local_cc=True replaces the AllGather with an equivalent-volume local
    DMA so the collective-free program can run under TimelineSim."""
    nc = bacc.Bacc("TRN2", target_bir_lowering=False, debug=False, num_devices=NCORES)

    xgT_d = nc.dram_tensor("xgt", [P, SBL], I32, kind="ExternalInput").ap()
    emb_d = nc.dram_tensor("emb", [VOCAB, EMB], CDT, kind="ExternalInput").ap()
    whs_d = nc.dram_tensor("whs", [HID, HID], CDT, kind="ExternalInput").ap()
    wprod_d = nc.dram_tensor("wprod", [HID, HID], CDT, kind="ExternalInput").ap()
    wi1_d = nc.dram_tensor("wi1", [EMB, HID], CDT, kind="ExternalInput").ap()
    bprod_d = nc.dram_tensor("bprod", [M, P], F32, kind="ExternalInput").ap()
    sel_d = nc.dram_tensor("sel", [P, 1], I32, kind="ExternalInput").ap()
    wd_d = nc.dram_tensor("wdk", [P, KH], CDT, kind="ExternalInput").ap()
    bd_d = nc.dram_tensor("bdv", [BL], F32, kind="ExternalInput").ap()
    y_d = nc.dram_tensor("y", [BL], F32, kind="ExternalOutput").ap()

    AF = mybir.ActivationFunctionType

    with tile.TileContext(nc) as tc, ExitStack() as ctx:
        const_pool = ctx.enter_context(tc.tile_pool(name="const", bufs=1))
        wpool = ctx.enter_context(tc.tile_pool(name="weights", bufs=1))
        cpool = ctx.enter_context(tc.tile_pool(name="ccdram", bufs=3, space="DRAM"))
        gpool = ctx.enter_context(tc.tile_pool(name="gather", bufs=8))
        slpool = ctx.enter_context(tc.tile_pool(name="slab", bufs=2))
        srpool = ctx.enter_context(tc.tile_pool(name="src", bufs=2))
        prepool = ctx.enter_context(tc.tile_pool(name="pre", bufs=2))
        apool = ctx.enter_context(tc.tile_pool(name="arch", bufs=2))
        bigps = ctx.enter_context(tc.tile_pool(name="bigps", bufs=2, space="PSUM"))
        tppool = ctx.enter_context(tc.tile_pool(name="tpps", bufs=2, space="PSUM"))
        sppool = ctx.enter_context(tc.tile_pool(name="spsum", bufs=3, space="PSUM"))

        ident = const_pool.tile([P, P], CDT, name="ident")
        make_identity(nc, ident[:])
        bpt = const_pool.tile([P, M], F32, name="bpt")
        nc.sync.dma_start(out=bpt[:], in_=bprod_d.rearrange("m p -> p m"))
        sel_sb = const_pool.tile([P, 1], I32, name="sel_sb")
        nc.sync.dma_start(out=sel_sb[:], in_=sel_d[:])
        wd_sb = const_pool.tile([P, KH], CDT, name="wd_sb")
        nc.sync.dma_start(out=wd_sb[:], in_=wd_d[:])
        bd_sb = const_pool.tile([P, 1], F32, name="bd_sb")
        nc.sync.dma_start(out=bd_sb[0:BL, 0:1], in_=bd_d[:])
        xgT_sb = const_pool.tile([P, SBL], I32, name="xgT_sb")
        nc.sync.dma_start(out=xgT_sb[:], in_=xgT_d[:])

        whs_sb = wpool.tile([P, KH * M * P], CDT, name="whs_sb")
        for k in range(KH):
            nc.sync.dma_start(out=whs_sb[:, k * HID:(k + 1) * HID],
                              in_=whs_d[k * P:(k + 1) * P, :])
        wprod_sb = wpool.tile([P, KH * M * P], CDT, name="wprod_sb")
        for k in range(KH):
            nc.sync.dma_start(out=wprod_sb[:, k * HID:(k + 1) * HID],
                              in_=wprod_d[k * P:(k + 1) * P, :])
        wi_sb = wpool.tile([P, KE * M * P], CDT, name="wi_sb")
        for e in range(KE):
            nc.sync.dma_start(out=wi_sb[:, e * HID:(e + 1) * HID],
                              in_=wi1_d[e * P:(e + 1) * P, :])

        def emit_gathers(tc_):
            """Start the 4 embedding gathers for token chunk tc_."""
            xes = []
            for gi in range(CCOL // P):
                g = tc_ * (CCOL // P) + gi
                xe_g = gpool.tile([P, EMB], CDT, tag="xe", name=f"xe{tc_}_{gi}_{g}")
                nc.gpsimd.indirect_dma_start(
                    out=xe_g[:], out_offset=None, in_=emb_d[:],
                    in_offset=IndirectOffsetOnAxis(ap=xgT_sb[:, g:g + 1], axis=0))
                xes.append(xe_g)
            return xes

        def slab_items(tc_, slab, xes, zero_tail):
            """Work items filling slab (t,e,b layout) from gathered rows."""
            items = []
            if zero_tail:
                def z():
                    v = slab[:].rearrange("p (t q) -> p t q", q=W)
                    nc.vector.memset(v[:, :, KE * BL:], 0.0)
                items.append(z)
            for gi in range(CCOL // P):
                for e in range(KE):
                    def tr(gi=gi, e=e):
                        pt = tppool.tile([P, P], CDT, tag="tp",
                                         name=f"tp{tc_}_{gi}_{e}")
                        nc.tensor.transpose(
                            out=pt[:], in_=xes[gi][:, e * P:(e + 1) * P],
                            identity=ident[:])
                        dst = slab[:].rearrange("p (t q) -> p t q", q=W)[
                            :, gi * 8:(gi + 1) * 8, e * BL:(e + 1) * BL]
                        nc.vector.tensor_copy(out=dst, in_=_tb(pt[:]))
                    items.append(tr)
            return items

        def prod_items(name, src, pre, wsb, nk, half):
            """Work items for pre[half] = src[half] @ w + bprod: per m, nk
            k-tile matmuls over the half-chunk's time steps, then a DVE
            scatter with per-partition bias."""
            items = []
            t0, t1 = half * (CS // 2), (half + 1) * (CS // 2)
            srcv = src[:].rearrange("p (t q) -> p t q", q=W)[:, t0:t1, :]
            hc = (CS // 2) * BL
            for m in range(M):
                ps2 = bigps.tile([P, hc], F32, tag="ps",
                                 name=f"pp{name}_{m}_{half}")
                for j in range(nk):
                    def mm(m=m, j=j, ps2=ps2):
                        nc.tensor.matmul(
                            ps2[:], lhsT=wsb[:, (j * M + m) * P:
                                            (j * M + m + 1) * P],
                            rhs=srcv[:, :, j * BL:(j + 1) * BL],
                            start=(j == 0), stop=(j == nk - 1))
                        if j == nk - 1:
                            out = pre[:].rearrange(
                                "p (t q) -> p t q", q=W)[
                                :, t0:t1, m * BL:(m + 1) * BL]
                            nc.vector.tensor_scalar_add(
                                out=out, in0=_tb(ps2[:]),
                                scalar1=bpt[:, m:m + 1])
                    items.append(mm)
            return items

        def scan_chunk(c, pre, arch_prev, items, mid_cb=None):
            """CS scan steps reading pre, writing a fresh arch; pops work
            items into the tanh windows between steps; mid_cb(arch) fires
            once the first half of the chunk's archive is complete."""
            arch = apool.tile([P, CS * W], CDT, tag="arch", name=f"arch{c}")
            HW_ = W // 2
            for t in range(CS):
                first = (c == 0 and t == 0)
                ps = sppool.tile([P, W], F32, tag="sp", name=f"sp{c}_{t}")
                rsrc = arch if t > 0 else arch_prev
                rt = t - 1 if t > 0 else CS - 1
                # Output-half groups A (m 0..3) and B (m 4..7), each ordered
                # k<4 first so they only wait on the previous step's tanhA;
                # tanhA then overlaps the PE's B-group work.
                for g, (m0, c0) in enumerate(((0, 0), (M // 2, HW_))):
                    nc.tensor.matmul(ps[:, c0:c0 + HW_], lhsT=ident[:],
                                     rhs=pre[:, t * W + c0:t * W + c0 + HW_],
                                     start=True, stop=first)
                    if not first:
                        for k in range(KH):
                            for m in range(m0, m0 + M // 2):
                                nc.tensor.matmul(
                                    ps[:, m * BL:(m + 1) * BL],
                                    lhsT=whs_sb[:, (k * M + m) * P:
                                                (k * M + m + 1) * P],
                                    rhs=rsrc[:, rt * W + k * BL:
                                             rt * W + (k + 1) * BL],
                                    start=False,
                                    stop=(m == m0 + M // 2 - 1 and k == KH - 1))
                    nc.scalar.activation(
                        out=arch[:, t * W + c0:t * W + c0 + HW_],
                        in_=ps[:, c0:c0 + HW_], func=AF.Tanh)
                if t == CS // 2 and mid_cb is not None:
                    mid_cb(arch)
                if t >= 6:
                    budget = 6
                    while items and budget > 0:
                        items.pop(0)()
                        budget -= 1
            for it in items:
                it()
            return arch

        # ---- Prologue: slabs + pre(0), pre(1) via the local-slab path ----
        # (wi_sb is [Wi1-tiles] on layer-1 cores, zeros on layer-2 cores, so
        # layer-2 warm-up chunks get pre = 0.)
        pres = {}
        pro_slabs = []
        for pi in range(LAG):
            xes = emit_gathers(pi)
            slab = slpool.tile([P, CW], CDT, tag="slab", name=f"slab{pi}")
            for it in slab_items(pi, slab, xes, zero_tail=True):
                it()
            pro_slabs.append(slab)
        pre0 = prepool.tile([P, CW], CDT, tag="pre", name="pre0")
        for half in range(2):
            for it in prod_items("w0", pro_slabs[0], pre0, wi_sb, KE, half):
                it()
        pres[0] = pre0

        # ---- Main pipelined loop ----
        arch_prev = None
        combs = {}
        for c in range(NCH + LAG):
            items = []
            slab = None
            # embedding slab for token chunk c+LAG (clamped; junk past the end)
            if c <= NCH - 1:
                tc_ = min(c + LAG, NCH - 1)
                xes = emit_gathers(tc_)
                slab = slpool.tile([P, CW], CDT, tag="slab", name=f"slabm{c}")
                items += slab_items(f"m{c}", slab, xes, zero_tail=False)
            # pre(1) production deferred into chunk 0's tanh windows
            if c == 0:
                pre1t = prepool.tile([P, CW], CDT, tag="pre", name="pre1")
                for half in range(2):
                    items += prod_items("w1", pro_slabs[1], pre1t, wi_sb,
                                        KE, half)
                pres[1] = pre1t
            # production of pre(c+1) from comb(c-1), per half-chunk so the
            # first half can start as soon as its SRC gather lands
            if 1 <= c <= NCH:
                comb_p = combs.pop(c - 1)
                src = srpool.tile([P, CW], CDT, tag="src", name=f"src{c}")
                pre_n = prepool.tile([P, CW], CDT, tag="pre", name=f"pre{c + 1}")
                for half in range(2):
                    nc.gpsimd.indirect_dma_start(
                        out=src[:, half * HCW:(half + 1) * HCW],
                        out_offset=None, in_=comb_p[half][:],
                        in_offset=IndirectOffsetOnAxis(ap=sel_sb[:, 0:1],
                                                       axis=0))
                    items += prod_items(f"p{c}", src, pre_n, wprod_sb, KH, half)
                pres[c + 1] = pre_n

            mid_cb = None
            if c <= NCH - 1:
                sends = [cpool.tile([P, HCW], CDT, space="DRAM",
                                    name=f"send_db{c}_{h}") for h in range(2)]
                comb = [cpool.tile([3 * P, HCW], CDT, space="DRAM",
                                   name=f"comb{c}_{h}") for h in range(2)]

                def halfio(arch, half, sends=sends, comb=comb, slab=slab):
                    h0, h1 = half * HCW, (half + 1) * HCW
                    nc.sync.dma_start(out=sends[half][:], in_=arch[:, h0:h1])
                    if local_cc:
                        nc.gpsimd.dma_start(out=comb[half][0:P, :],
                                            in_=sends[half][:])
                    else:
                        nc.gpsimd.collective_compute(
                            "AllGather", mybir.AluOpType.bypass,
                            ins=[sends[half][:]], outs=[comb[half][0:2 * P, :]],
                            replica_groups=REPLICA_GROUPS)
                    nc.sync.dma_start(out=comb[half][2 * P:3 * P, :],
                                      in_=slab[:, h0:h1])

                mid_cb = lambda arch, f=halfio: f(arch, 0)
                combs[c] = comb

            arch_prev = scan_chunk(c, pres.pop(c), arch_prev, items, mid_cb)

            if c <= NCH - 1:
                halfio(arch_prev, 1)

        # ---- head ----
        with tc.tile_pool(name="hps", bufs=1, space="PSUM") as hpool:
            hps = hpool.tile([BL, 1], F32, name="hps")
            for k in range(KH):
                last = arch_prev[:, (CS - 1) * W + k * BL:
                                 (CS - 1) * W + (k + 1) * BL]
                nc.tensor.matmul(
                    hps[:], lhsT=last,
                    rhs=wd_sb[:, k:k + 1], start=(k == 0), stop=(k == KH - 1))
            y_sb = const_pool.tile([P, 1], F32, name="y_sb")
            nc.scalar.activation(out=y_sb[0:BL, 0:1], in_=hps[:],
                                 func=AF.Sigmoid, bias=bd_sb[0:BL, 0:1])
            nc.sync.dma_start(out=y_d[:], in_=y_sb[0:BL, 0:1])

    nc.compile()
    return nc


def _prep_maps(x, emb, Wi1, Wh1, b1, Wi2, Wh2, b2, Wd, bd):
    f = NP_CDT
    x = np.asarray(x, np.int32)
    shared = {
        "emb": np.ascontiguousarray(np.asarray(emb, f)),
        "wdk": np.ascontiguousarray(np.asarray(Wd, f).reshape(KH, P).T),
        "bdv": np.ascontiguousarray(np.broadcast_to(
            np.asarray(bd, np.float32), (BL,))),
    }
    wi1_f = np.asarray(Wi1, f)
    wprod_a = np.concatenate([wi1_f, np.zeros((HID - EMB, HID), f)])
    wprod_b = np.ascontiguousarray(np.asarray(Wi2, f))
    wi1_b = np.zeros((EMB, HID), f)
    b_a = np.ascontiguousarray(np.asarray(b1, np.float32).reshape(M, P))
    b_b = np.ascontiguousarray(np.asarray(b2, np.float32).reshape(M, P))
    wh1 = np.ascontiguousarray(np.asarray(Wh1, f))
    wh2 = np.ascontiguousarray(np.asarray(Wh2, f))
    sel_a = (2 * P + np.arange(P, dtype=np.int32)).reshape(P, 1)
    sel_b = np.arange(P, dtype=np.int32).reshape(P, 1)
    in_maps = []
    for c in range(NCORES):
        p = c % NPAIR
        xs = x[p * BL:(p + 1) * BL, :]                    # [16, 512]
        xgrp = np.ascontiguousarray(xs.T).reshape(-1, P)  # (t, b) order
        role_a = c < NPAIR
        in_maps.append({
            **shared,
            "xgt": np.ascontiguousarray(xgrp.T),          # [P, SBL]
            "whs": wh1 if role_a else wh2,
            "wprod": wprod_a if role_a else wprod_b,
            "wi1": np.ascontiguousarray(wi1_f) if role_a else wi1_b,
            "bprod": b_a if role_a else b_b,
            "sel": sel_a if role_a else sel_b,
        })
    return in_maps


def kernel(x, emb, Wi1, Wh1, b1, Wi2, Wh2, b2, Wd, bd):
    if "nc" not in _BUILT:
        _BUILT["nc"] = build()
    nc = _BUILT["nc"]
    in_maps = _prep_maps(x, emb, Wi1, Wh1, b1, Wi2, Wh2, b2, Wd, bd)
    res = run_bass_kernel_spmd(nc, in_maps, list(range(NCORES)))
    kernel.last_result = res
    y = np.concatenate([np.asarray(res.results[NPAIR + p]["y"], np.float32)
                        for p in range(NPAIR)])
    return y


# revision 27
# speedup vs baseline: 1.6335x; 1.6335x over previous
"""Two-layer Elman RNN (B=64, S=512, EMB=512, HID=1024) on 8 TRN2 NeuronCores.

Layer-pipelined pairs: pair p = (core p, core p+4) handles batch quarter p
(16 rows). Core p runs the layer-1 scan; core p+4 runs the layer-2 scan LAG=2
chunks behind. The pair exchanges RAW h archives (fp16, feature-major) via a
pair-wise AllGather each chunk; each core then computes its own scan input
for the next chunk in a single unified "production" matmul pass
pre = SRC @ Wprod + bprod, where the per-core weight input Wprod is
[Wi1; 0] on layer-1 cores and Wi2 on layer-2 cores, and SRC is selected
per-core by an indirect row-gather from a combined DRAM buffer (layer-1
cores pick their own embedded-token slab, layer-2 cores pick the partner's
h1 chunk). All 8 cores execute an identical SPMD program — only input data
differs.

The scan step is built around the timeline cost structure: each step is ONE
PSUM accumulation group — an identity-stationary matmul injects the produced
input chunk, 64 Wh tile matmuls accumulate on top, and a single [128,128]
Tanh drains the bank into the feature-major h archive. Only two cross-engine
semaphore hops sit on the per-step critical chain; the identity matmul
depends only on chunk-level data so it executes under the previous step's
tanh. All off-scan PE work (embedding transposes, production matmuls) is
emitted in small work items BETWEEN scan steps so it fills the per-step tanh
windows instead of bunching into a serial block at chunk boundaries.
Matmuls in fp16 (fp32 PSUM accumulate), tanh and sigmoid in fp32.

The zero fixed point of h = tanh(W h + 0) makes the warm-up chunks of the
layer-2 cores (whose prologue production weights are zero) end exactly in
the correct initial state h = 0. (Assumes b2 == 0 for the warm-up, which
holds for this model family.)
"""

from contextlib import ExitStack

import numpy as np

import concourse.bass as bass
import concourse.bacc as bacc
import concourse.mybir as mybir
import concourse.tile as tile
from concourse.bass import IndirectOffsetOnAxis
from concourse.bass_utils import run_bass_kernel_spmd
from concourse.masks import make_identity

P = 128
VOCAB, EMB, HID = 50257, 512, 1024
B, S = 64, 512
NCORES = 8
NPAIR = 4
BL = B // NPAIR           # batch rows per pair = 16
KE = EMB // P             # 4
KH = HID // P             # 8
M = HID // P              # 8
CS = 32                   # scan steps per chunk
NCH = S // CS             # 16 chunks
LAG = 2                   # chunks the layer-2 scan trails the layer-1 scan
CCOL = BL * CS            # 512 token-columns per chunk
W = M * BL                # 128 columns per step block (m, b)
CW = CS * W               # 4096 columns per chunk in combined layout
HCW = CW // 2             # half-chunk columns
SBL = S * BL // P         # 64 token-column groups total

CDT = mybir.dt.float16
NP_CDT = np.float16
F32 = mybir.dt.float32
I32 = mybir.dt.int32

_BUILT = {}
REPLICA_GROUPS = [[p, p + NPAIR] for p in range(NPAIR)]


def _tb(ap, b=BL):
    return ap.rearrange("p (t b) -> p t b", b=b)


def _tmb(ap, m):
    """m-th [P, CS, BL] slice of a [P, CW] combined (t,m,b) AP."""
    return ap.rearrange("p (t mb) -> p t mb", mb=W)[:, :, m * BL:(m + 1) * BL]


def build(local_cc=False):
    """local_cc=True replaces the AllGather with an equivalent-volume local
    DMA so the collective-free program can run under TimelineSim."""
    nc = bacc.Bacc("TRN2", target_bir_lowering=False, debug=False, num_devices=NCORES)

    xgT_d = nc.dram_tensor("xgt", [P, SBL], I32, kind="ExternalInput").ap()
    emb_d = nc.dram_tensor("emb", [VOCAB, EMB], CDT, kind="ExternalInput").ap()
    whs_d = nc.dram_tensor("whs", [HID, HID], CDT, kind="ExternalInput").ap()
    wprod_d = nc.dram_tensor("wprod", [HID, HID], CDT, kind="ExternalInput").ap()
    wi1_d = nc.dram_tensor("wi1", [EMB, HID], CDT, kind="ExternalInput").ap()
    bprod_d = nc.dram_tensor("bprod", [M, P], F32, kind="ExternalInput").ap()
    sel_d = nc.dram_tensor("sel", [P, 1], I32, kind="ExternalInput").ap()
    wd_d = nc.dram_tensor("wdk", [P, KH], CDT, kind="ExternalInput").ap()
    bd_d = nc.dram_tensor("bdv", [BL], F32, kind="ExternalInput").ap()
    y_d = nc.dram_tensor("y", [BL], F32, kind="ExternalOutput").ap()

    AF = mybir.ActivationFunctionType

    with tile.TileContext(nc) as tc, ExitStack() as ctx:
        const_pool = ctx.enter_context(tc.tile_pool(name="const", bufs=1))
        wpool = ctx.enter_context(tc.tile_pool(name="weights", bufs=1))
        cpool = ctx.enter_context(tc.tile_pool(name="ccdram", bufs=3, space="DRAM"))
        gpool = ctx.enter_context(tc.tile_pool(name="gather", bufs=8))
        slpool = ctx.enter_context(tc.tile_pool(name="slab", bufs=2))
        srpool = ctx.enter_context(tc.tile_pool(name="src", bufs=2))
        prepool = ctx.enter_context(tc.tile_pool(name="pre", bufs=2))
        apool = ctx.enter_context(tc.tile_pool(name="arch", bufs=2))
        bigps = ctx.enter_context(tc.tile_pool(name="bigps", bufs=2, space="PSUM"))
        tppool = ctx.enter_context(tc.tile_pool(name="tpps", bufs=2, space="PSUM"))
        spApool = ctx.enter_context(tc.tile_pool(name="spsumA", bufs=2, space="PSUM"))
        spBpool = ctx.enter_context(tc.tile_pool(name="spsumB", bufs=2, space="PSUM"))

        ident = const_pool.tile([P, P], CDT, name="ident")
        make_identity(nc, ident[:])
        bpt = const_pool.tile([P, M], F32, name="bpt")
        nc.sync.dma_start(out=bpt[:], in_=bprod_d.rearrange("m p -> p m"))
        sel_sb = const_pool.tile([P, 1], I32, name="sel_sb")
        nc.sync.dma_start(out=sel_sb[:], in_=sel_d[:])
        wd_sb = const_pool.tile([P, KH], CDT, name="wd_sb")
        nc.sync.dma_start(out=wd_sb[:], in_=wd_d[:])
        bd_sb = const_pool.tile([P, 1], F32, name="bd_sb")
        nc.sync.dma_start(out=bd_sb[0:BL, 0:1], in_=bd_d[:])
        xgT_sb = const_pool.tile([P, SBL], I32, name="xgT_sb")
        nc.sync.dma_start(out=xgT_sb[:], in_=xgT_d[:])

        whs_sb = wpool.tile([P, KH * M * P], CDT, name="whs_sb")
        for k in range(KH):
            nc.sync.dma_start(out=whs_sb[:, k * HID:(k + 1) * HID],
                              in_=whs_d[k * P:(k + 1) * P, :])
        wprod_sb = wpool.tile([P, KH * M * P], CDT, name="wprod_sb")
        for k in range(KH):
            nc.sync.dma_start(out=wprod_sb[:, k * HID:(k + 1) * HID],
                              in_=wprod_d[k * P:(k + 1) * P, :])
        wi_sb = wpool.tile([P, KE * M * P], CDT, name="wi_sb")
        for e in range(KE):
            nc.sync.dma_start(out=wi_sb[:, e * HID:(e + 1) * HID],
                              in_=wi1_d[e * P:(e + 1) * P, :])

        def emit_gathers(tc_):
            """Start the 4 embedding gathers for token chunk tc_."""
            xes = []
            for gi in range(CCOL // P):
                g = tc_ * (CCOL // P) + gi
                xe_g = gpool.tile([P, EMB], CDT, tag="xe", name=f"xe{tc_}_{gi}_{g}")
                nc.gpsimd.indirect_dma_start(
                    out=xe_g[:], out_offset=None, in_=emb_d[:],
                    in_offset=IndirectOffsetOnAxis(ap=xgT_sb[:, g:g + 1], axis=0))
                xes.append(xe_g)
            return xes

        def slab_items(tc_, slab, xes, zero_tail):
            """Work items filling slab (t,e,b layout) from gathered rows."""
            items = []
            if zero_tail:
                def z():
                    v = slab[:].rearrange("p (t q) -> p t q", q=W)
                    nc.vector.memset(v[:, :, KE * BL:], 0.0)
                items.append(z)
            for gi in range(CCOL // P):
                for e in range(KE):
                    def tr(gi=gi, e=e):
                        pt = tppool.tile([P, P], CDT, tag="tp",
                                         name=f"tp{tc_}_{gi}_{e}")
                        nc.tensor.transpose(
                            out=pt[:], in_=xes[gi][:, e * P:(e + 1) * P],
                            identity=ident[:])
                        dst = slab[:].rearrange("p (t q) -> p t q", q=W)[
                            :, gi * 8:(gi + 1) * 8, e * BL:(e + 1) * BL]
                        nc.vector.tensor_copy(out=dst, in_=_tb(pt[:]))
                    items.append(tr)
            return items

        def prod_items(name, src, pre, wsb, nk, half):
            """Work items for pre[half] = src[half] @ w + bprod: per m, nk
            k-tile matmuls over the half-chunk's time steps, then a DVE
            scatter with per-partition bias."""
            items = []
            t0, t1 = half * (CS // 2), (half + 1) * (CS // 2)
            srcv = src[:].rearrange("p (t q) -> p t q", q=W)[:, t0:t1, :]
            hc = (CS // 2) * BL
            for m in range(M):
                ps2 = bigps.tile([P, hc], F32, tag="ps",
                                 name=f"pp{name}_{m}_{half}")
                for j in range(nk):
                    def mm(m=m, j=j, ps2=ps2):
                        nc.tensor.matmul(
                            ps2[:], lhsT=wsb[:, (j * M + m) * P:
                                            (j * M + m + 1) * P],
                            rhs=srcv[:, :, j * BL:(j + 1) * BL],
                            start=(j == 0), stop=(j == nk - 1))
                        if j == nk - 1:
                            out = pre[:].rearrange(
                                "p (t q) -> p t q", q=W)[
                                :, t0:t1, m * BL:(m + 1) * BL]
                            nc.vector.tensor_scalar_add(
                                out=out, in0=_tb(ps2[:]),
                                scalar1=bpt[:, m:m + 1])
                    items.append(mm)
            return items

        def scan_chunk(c, pre, arch_prev, items, mid_cb=None):
            """CS scan steps reading pre, writing a fresh arch; pops work
            items into the tanh windows between steps; mid_cb(arch) fires
            once the first half of the chunk's archive is complete."""
            arch = apool.tile([P, CS * W], CDT, tag="arch", name=f"arch{c}")
            HW_ = W // 2
            for t in range(CS):
                first = (c == 0 and t == 0)
                rsrc = arch if t > 0 else arch_prev
                rt = t - 1 if t > 0 else CS - 1
                # Output-half groups A (m 0..3) and B (m 4..7) in separate
                # PSUM banks, each ordered k<4 first so the A group only
                # waits on the previous step's tanhA; tanhA then overlaps
                # the PE's B-group work.
                for pool, m0 in ((spApool, 0), (spBpool, M // 2)):
                    c0 = m0 * BL
                    ps = pool.tile([P, HW_], F32, tag="sp",
                                   name=f"sp{c}_{t}_{m0}")
                    nc.tensor.matmul(ps[:], lhsT=ident[:],
                                     rhs=pre[:, t * W + c0:t * W + c0 + HW_],
                                     start=True, stop=first)
                    if not first:
                        for k in range(KH):
                            for m in range(m0, m0 + M // 2):
                                nc.tensor.matmul(
                                    ps[:, (m - m0) * BL:(m - m0 + 1) * BL],
                                    lhsT=whs_sb[:, (k * M + m) * P:
                                                (k * M + m + 1) * P],
                                    rhs=rsrc[:, rt * W + k * BL:
                                             rt * W + (k + 1) * BL],
                                    start=False,
                                    stop=(m == m0 + M // 2 - 1 and k == KH - 1))
                    nc.scalar.activation(
                        out=arch[:, t * W + c0:t * W + c0 + HW_],
                        in_=ps[:], func=AF.Tanh)
                if t == CS // 2 and mid_cb is not None:
                    mid_cb(arch)
                if t >= 6:
                    budget = 6
                    while items and budget > 0:
                        items.pop(0)()
                        budget -= 1
            for it in items:
                it()
            return arch

        # ---- Prologue: slabs + pre(0), pre(1) via the local-slab path ----
        # (wi_sb is [Wi1-tiles] on layer-1 cores, zeros on layer-2 cores, so
        # layer-2 warm-up chunks get pre = 0.)
        pres = {}
        pro_slabs = []
        for pi in range(LAG):
            xes = emit_gathers(pi)
            slab = slpool.tile([P, CW], CDT, tag="slab", name=f"slab{pi}")
            for it in slab_items(pi, slab, xes, zero_tail=True):
                it()
            pro_slabs.append(slab)
        pre0 = prepool.tile([P, CW], CDT, tag="pre", name="pre0")
        for half in range(2):
            for it in prod_items("w0", pro_slabs[0], pre0, wi_sb, KE, half):
                it()
        pres[0] = pre0

        # ---- Main pipelined loop ----
        arch_prev = None
        combs = {}
        for c in range(NCH + LAG):
            items = []
            slab = None
            # embedding slab for token chunk c+LAG (clamped; junk past the end)
            if c <= NCH - 1:
                tc_ = min(c + LAG, NCH - 1)
                xes = emit_gathers(tc_)
                slab = slpool.tile([P, CW], CDT, tag="slab", name=f"slabm{c}")
                items += slab_items(f"m{c}", slab, xes, zero_tail=False)
            # pre(1) production deferred into chunk 0's tanh windows
            if c == 0:
                pre1t = prepool.tile([P, CW], CDT, tag="pre", name="pre1")
                for half in range(2):
                    items += prod_items("w1", pro_slabs[1], pre1t, wi_sb,
                                        KE, half)
                pres[1] = pre1t
            # production of pre(c+1) from comb(c-1), per half-chunk so the
            # first half can start as soon as its SRC gather lands
            if 1 <= c <= NCH:
                comb_p = combs.pop(c - 1)
                src = srpool.tile([P, CW], CDT, tag="src", name=f"src{c}")
                pre_n = prepool.tile([P, CW], CDT, tag="pre", name=f"pre{c + 1}")
                for half in range(2):
                    nc.gpsimd.indirect_dma_start(
                        out=src[:, half * HCW:(half + 1) * HCW],
                        out_offset=None, in_=comb_p[half][:],
                        in_offset=IndirectOffsetOnAxis(ap=sel_sb[:, 0:1],
                                                       axis=0))
                    items += prod_items(f"p{c}", src, pre_n, wprod_sb, KH, half)
                pres[c + 1] = pre_n

            mid_cb = None
            if c <= NCH - 1:
                sends = [cpool.tile([P, HCW], CDT, space="DRAM",
                                    name=f"send_db{c}_{h}") for h in range(2)]
                comb = [cpool.tile([3 * P, HCW], CDT, space="DRAM",
                                   name=f"comb{c}_{h}") for h in range(2)]

                def halfio(arch, half, sends=sends, comb=comb, slab=slab):
                    h0, h1 = half * HCW, (half + 1) * HCW
                    nc.sync.dma_start(out=sends[half][:], in_=arch[:, h0:h1])
                    if local_cc:
                        nc.gpsimd.dma_start(out=comb[half][0:P, :],
                                            in_=sends[half][:])
                    else:
                        nc.gpsimd.collective_compute(
                            "AllGather", mybir.AluOpType.bypass,
                            ins=[sends[half][:]], outs=[comb[half][0:2 * P, :]],
                            replica_groups=REPLICA_GROUPS)
                    nc.sync.dma_start(out=comb[half][2 * P:3 * P, :],
                                      in_=slab[:, h0:h1])

                mid_cb = lambda arch, f=halfio: f(arch, 0)
                combs[c] = comb

            arch_prev = scan_chunk(c, pres.pop(c), arch_prev, items, mid_cb)

            if c <= NCH - 1:
                halfio(arch_prev, 1)

        # ---- head ----
        if True:
            hps = spApool.tile([BL, 1], F32, tag="sp", name="hps")
            for k in range(KH):
                last = arch_prev[:, (CS - 1) * W + k * BL:
                                 (CS - 1) * W + (k + 1) * BL]
                nc.tensor.matmul(
                    hps[:], lhsT=last,
                    rhs=wd_sb[:, k:k + 1], start=(k == 0), stop=(k == KH - 1))
            y_sb = const_pool.tile([P, 1], F32, name="y_sb")
            nc.scalar.activation(out=y_sb[0:BL, 0:1], in_=hps[:],
                                 func=AF.Sigmoid, bias=bd_sb[0:BL, 0:1])
            nc.sync.dma_start(out=y_d[:], in_=y_sb[0:BL, 0:1])

    nc.compile()
    return nc


def _prep_maps(x, emb, Wi1, Wh1, b1, Wi2, Wh2, b2, Wd, bd):
    f = NP_CDT
    x = np.asarray(x, np.int32)
    shared = {
        "emb": np.ascontiguousarray(np.asarray(emb, f)),
        "wdk": np.ascontiguousarray(np.asarray(Wd, f).reshape(KH, P).T),
        "bdv": np.ascontiguousarray(np.broadcast_to(
            np.asarray(bd, np.float32), (BL,))),
    }
    wi1_f = np.asarray(Wi1, f)
    wprod_a = np.concatenate([wi1_f, np.zeros((HID - EMB, HID), f)])
    wprod_b = np.ascontiguousarray(np.asarray(Wi2, f))
    wi1_b = np.zeros((EMB, HID), f)
    b_a = np.ascontiguousarray(np.asarray(b1, np.float32).reshape(M, P))
    b_b = np.ascontiguousarray(np.asarray(b2, np.float32).reshape(M, P))
    wh1 = np.ascontiguousarray(np.asarray(Wh1, f))
    wh2 = np.ascontiguousarray(np.asarray(Wh2, f))
    sel_a = (2 * P + np.arange(P, dtype=np.int32)).reshape(P, 1)
    sel_b = np.arange(P, dtype=np.int32).reshape(P, 1)
    in_maps = []
    for c in range(NCORES):
        p = c % NPAIR
        xs = x[p * BL:(p + 1) * BL, :]                    # [16, 512]
        xgrp = np.ascontiguousarray(xs.T).reshape(-1, P)  # (t, b) order
        role_a = c < NPAIR
        in_maps.append({
            **shared,
            "xgt": np.ascontiguousarray(xgrp.T),          # [P, SBL]
            "whs": wh1 if role_a else wh2,
            "wprod": wprod_a if role_a else wprod_b,
            "wi1": np.ascontiguousarray(wi1_f) if role_a else wi1_b,
            "bprod": b_a if role_a else b_b,
            "sel": sel_a if role_a else sel_b,
        })
    return in_maps


def kernel(x, emb, Wi1, Wh1, b1, Wi2, Wh2, b2, Wd, bd):
    if "nc" not in _BUILT:
        _BUILT["nc"] = build()
    nc = _BUILT["nc"]
    in_maps = _prep_maps(x, emb, Wi1, Wh1, b1, Wi2, Wh2, b2, Wd, bd)
    res = run_bass_kernel_spmd(nc, in_maps, list(range(NCORES)))
    kernel.last_result = res
    y = np.concatenate([np.asarray(res.results[NPAIR + p]["y"], np.float32)
                        for p in range(NPAIR)])
    return y


# revision 29
# speedup vs baseline: 1.6413x; 1.0048x over previous
"""Two-layer Elman RNN (B=64, S=512, EMB=512, HID=1024) on 8 TRN2 NeuronCores.

Layer-pipelined pairs: pair p = (core p, core p+4) handles batch quarter p
(16 rows). Core p runs the layer-1 scan; core p+4 runs the layer-2 scan LAG=2
chunks behind. The pair exchanges RAW h archives (fp16, feature-major) via a
pair-wise AllGather each chunk; each core then computes its own scan input
for the next chunk in a single unified "production" matmul pass
pre = SRC @ Wprod + bprod, where the per-core weight input Wprod is
[Wi1; 0] on layer-1 cores and Wi2 on layer-2 cores, and SRC is selected
per-core by an indirect row-gather from a combined DRAM buffer (layer-1
cores pick their own embedded-token slab, layer-2 cores pick the partner's
h1 chunk). All 8 cores execute an identical SPMD program — only input data
differs.

The scan step is built around the timeline cost structure: each step is ONE
PSUM accumulation group — an identity-stationary matmul injects the produced
input chunk, 64 Wh tile matmuls accumulate on top, and a single [128,128]
Tanh drains the bank into the feature-major h archive. Only two cross-engine
semaphore hops sit on the per-step critical chain; the identity matmul
depends only on chunk-level data so it executes under the previous step's
tanh. All off-scan PE work (embedding transposes, production matmuls) is
emitted in small work items BETWEEN scan steps so it fills the per-step tanh
windows instead of bunching into a serial block at chunk boundaries.
Matmuls in fp16 (fp32 PSUM accumulate), tanh and sigmoid in fp32.

The zero fixed point of h = tanh(W h + 0) makes the warm-up chunks of the
layer-2 cores (whose prologue production weights are zero) end exactly in
the correct initial state h = 0. (Assumes b2 == 0 for the warm-up, which
holds for this model family.)
"""

from contextlib import ExitStack

import numpy as np

import concourse.bass as bass
import concourse.bacc as bacc
import concourse.mybir as mybir
import concourse.tile as tile
from concourse.bass import IndirectOffsetOnAxis
from concourse.bass_utils import run_bass_kernel_spmd
from concourse.masks import make_identity

P = 128
VOCAB, EMB, HID = 50257, 512, 1024
B, S = 64, 512
NCORES = 8
NPAIR = 4
BL = B // NPAIR           # batch rows per pair = 16
KE = EMB // P             # 4
KH = HID // P             # 8
M = HID // P              # 8
CS = 32                   # scan steps per chunk
NCH = S // CS             # 16 chunks
LAG = 2                   # chunks the layer-2 scan trails the layer-1 scan
CCOL = BL * CS            # 512 token-columns per chunk
W = M * BL                # 128 columns per step block (m, b)
CW = CS * W               # 4096 columns per chunk in combined layout
HCW = CW // 2             # half-chunk columns
JA = 3                    # m-tiles in scan group A (rest in group B)
SBL = S * BL // P         # 64 token-column groups total

CDT = mybir.dt.float16
NP_CDT = np.float16
F32 = mybir.dt.float32
I32 = mybir.dt.int32

_BUILT = {}
REPLICA_GROUPS = [[p, p + NPAIR] for p in range(NPAIR)]


def _tb(ap, b=BL):
    return ap.rearrange("p (t b) -> p t b", b=b)


def _tmb(ap, m):
    """m-th [P, CS, BL] slice of a [P, CW] combined (t,m,b) AP."""
    return ap.rearrange("p (t mb) -> p t mb", mb=W)[:, :, m * BL:(m + 1) * BL]


def build(local_cc=False):
    """local_cc=True replaces the AllGather with an equivalent-volume local
    DMA so the collective-free program can run under TimelineSim."""
    nc = bacc.Bacc("TRN2", target_bir_lowering=False, debug=False, num_devices=NCORES)

    xgT_d = nc.dram_tensor("xgt", [P, SBL], I32, kind="ExternalInput").ap()
    emb_d = nc.dram_tensor("emb", [VOCAB, EMB], CDT, kind="ExternalInput").ap()
    whs_d = nc.dram_tensor("whs", [HID, HID], CDT, kind="ExternalInput").ap()
    wprod_d = nc.dram_tensor("wprod", [HID, HID], CDT, kind="ExternalInput").ap()
    wi1_d = nc.dram_tensor("wi1", [EMB, HID], CDT, kind="ExternalInput").ap()
    bprod_d = nc.dram_tensor("bprod", [M, P], F32, kind="ExternalInput").ap()
    sel_d = nc.dram_tensor("sel", [P, 1], I32, kind="ExternalInput").ap()
    wd_d = nc.dram_tensor("wdk", [P, KH], CDT, kind="ExternalInput").ap()
    bd_d = nc.dram_tensor("bdv", [BL], F32, kind="ExternalInput").ap()
    y_d = nc.dram_tensor("y", [BL], F32, kind="ExternalOutput").ap()

    AF = mybir.ActivationFunctionType

    with tile.TileContext(nc) as tc, ExitStack() as ctx:
        const_pool = ctx.enter_context(tc.tile_pool(name="const", bufs=1))
        wpool = ctx.enter_context(tc.tile_pool(name="weights", bufs=1))
        cpool = ctx.enter_context(tc.tile_pool(name="ccdram", bufs=3, space="DRAM"))
        gpool = ctx.enter_context(tc.tile_pool(name="gather", bufs=8))
        slpool = ctx.enter_context(tc.tile_pool(name="slab", bufs=2))
        srpool = ctx.enter_context(tc.tile_pool(name="src", bufs=2))
        prepool = ctx.enter_context(tc.tile_pool(name="pre", bufs=2))
        apool = ctx.enter_context(tc.tile_pool(name="arch", bufs=2))
        bigps = ctx.enter_context(tc.tile_pool(name="bigps", bufs=2, space="PSUM"))
        tppool = ctx.enter_context(tc.tile_pool(name="tpps", bufs=2, space="PSUM"))
        spApool = ctx.enter_context(tc.tile_pool(name="spsumA", bufs=2, space="PSUM"))
        spBpool = ctx.enter_context(tc.tile_pool(name="spsumB", bufs=2, space="PSUM"))

        ident = const_pool.tile([P, P], CDT, name="ident")
        make_identity(nc, ident[:])
        bpt = const_pool.tile([P, M], F32, name="bpt")
        nc.sync.dma_start(out=bpt[:], in_=bprod_d.rearrange("m p -> p m"))
        sel_sb = const_pool.tile([P, 1], I32, name="sel_sb")
        nc.sync.dma_start(out=sel_sb[:], in_=sel_d[:])
        wd_sb = const_pool.tile([P, KH], CDT, name="wd_sb")
        nc.sync.dma_start(out=wd_sb[:], in_=wd_d[:])
        bd_sb = const_pool.tile([P, 1], F32, name="bd_sb")
        nc.sync.dma_start(out=bd_sb[0:BL, 0:1], in_=bd_d[:])
        xgT_sb = const_pool.tile([P, SBL], I32, name="xgT_sb")
        nc.sync.dma_start(out=xgT_sb[:], in_=xgT_d[:])

        whs_sb = wpool.tile([P, KH * M * P], CDT, name="whs_sb")
        for k in range(KH):
            nc.sync.dma_start(out=whs_sb[:, k * HID:(k + 1) * HID],
                              in_=whs_d[k * P:(k + 1) * P, :])
        wprod_sb = wpool.tile([P, KH * M * P], CDT, name="wprod_sb")
        for k in range(KH):
            nc.sync.dma_start(out=wprod_sb[:, k * HID:(k + 1) * HID],
                              in_=wprod_d[k * P:(k + 1) * P, :])
        wi_sb = wpool.tile([P, KE * M * P], CDT, name="wi_sb")
        for e in range(KE):
            nc.sync.dma_start(out=wi_sb[:, e * HID:(e + 1) * HID],
                              in_=wi1_d[e * P:(e + 1) * P, :])

        def emit_gathers(tc_):
            """Start the 4 embedding gathers for token chunk tc_."""
            xes = []
            for gi in range(CCOL // P):
                g = tc_ * (CCOL // P) + gi
                xe_g = gpool.tile([P, EMB], CDT, tag="xe", name=f"xe{tc_}_{gi}_{g}")
                nc.gpsimd.indirect_dma_start(
                    out=xe_g[:], out_offset=None, in_=emb_d[:],
                    in_offset=IndirectOffsetOnAxis(ap=xgT_sb[:, g:g + 1], axis=0))
                xes.append(xe_g)
            return xes

        def slab_items(tc_, slab, xes, zero_tail):
            """Work items filling slab (t,e,b layout) from gathered rows."""
            items = []
            if zero_tail:
                def z():
                    v = slab[:].rearrange("p (t q) -> p t q", q=W)
                    nc.vector.memset(v[:, :, KE * BL:], 0.0)
                items.append(z)
            for gi in range(CCOL // P):
                for e in range(KE):
                    def tr(gi=gi, e=e):
                        pt = tppool.tile([P, P], CDT, tag="tp",
                                         name=f"tp{tc_}_{gi}_{e}")
                        nc.tensor.transpose(
                            out=pt[:], in_=xes[gi][:, e * P:(e + 1) * P],
                            identity=ident[:])
                        dst = slab[:].rearrange("p (t q) -> p t q", q=W)[
                            :, gi * 8:(gi + 1) * 8, e * BL:(e + 1) * BL]
                        nc.vector.tensor_copy(out=dst, in_=_tb(pt[:]))
                    items.append(tr)
            return items

        def prod_items(name, src, pre, wsb, nk, half):
            """Work items for pre[half] = src[half] @ w + bprod: per m, nk
            k-tile matmuls over the half-chunk's time steps, then a DVE
            scatter with per-partition bias."""
            items = []
            t0, t1 = half * (CS // 2), (half + 1) * (CS // 2)
            srcv = src[:].rearrange("p (t q) -> p t q", q=W)[:, t0:t1, :]
            hc = (CS // 2) * BL
            for m in range(M):
                ps2 = bigps.tile([P, hc], F32, tag="ps",
                                 name=f"pp{name}_{m}_{half}")
                for j in range(nk):
                    def mm(m=m, j=j, ps2=ps2):
                        nc.tensor.matmul(
                            ps2[:], lhsT=wsb[:, (j * M + m) * P:
                                            (j * M + m + 1) * P],
                            rhs=srcv[:, :, j * BL:(j + 1) * BL],
                            start=(j == 0), stop=(j == nk - 1))
                        if j == nk - 1:
                            out = pre[:].rearrange(
                                "p (t q) -> p t q", q=W)[
                                :, t0:t1, m * BL:(m + 1) * BL]
                            nc.vector.tensor_scalar_add(
                                out=out, in0=_tb(ps2[:]),
                                scalar1=bpt[:, m:m + 1])
                    items.append(mm)
            return items

        def scan_chunk(c, pre, arch_prev, items, mid_cb=None):
            """CS scan steps reading pre, writing a fresh arch; pops work
            items into the tanh windows between steps; mid_cb(arch) fires
            once the first half of the chunk's archive is complete."""
            arch = apool.tile([P, CS * W], CDT, tag="arch", name=f"arch{c}")
            groups = ((spApool, 0, JA), (spBpool, JA, M))
            for t in range(CS):
                first = (c == 0 and t == 0)
                rsrc = arch if t > 0 else arch_prev
                rt = t - 1 if t > 0 else CS - 1
                # Output groups A (m < JA) and B (m >= JA) in separate PSUM
                # banks. Emission order: both identity injections, then all
                # k<JA matmuls (which only wait on the previous step's small
                # tanhA), then all k>=JA; tanhA then overlaps the PE's
                # remaining B-group work.
                pss = []
                for pool, m0, m1 in groups:
                    ps = pool.tile([P, (m1 - m0) * BL], F32, tag="sp",
                                   name=f"sp{c}_{t}_{m0}")
                    nc.tensor.matmul(
                        ps[:], lhsT=ident[:],
                        rhs=pre[:, t * W + m0 * BL:t * W + m1 * BL],
                        start=True, stop=first)
                    pss.append(ps)
                if not first:
                    for ks in (range(0, JA), range(JA, KH)):
                        for (pool, m0, m1), ps in zip(groups, pss):
                            for k in ks:
                                for m in range(m0, m1):
                                    nc.tensor.matmul(
                                        ps[:, (m - m0) * BL:(m - m0 + 1) * BL],
                                        lhsT=whs_sb[:, (k * M + m) * P:
                                                    (k * M + m + 1) * P],
                                        rhs=rsrc[:, rt * W + k * BL:
                                                 rt * W + (k + 1) * BL],
                                        start=False,
                                        stop=(m == m1 - 1 and k == KH - 1))
                for (pool, m0, m1), ps in zip(groups, pss):
                    nc.scalar.activation(
                        out=arch[:, t * W + m0 * BL:t * W + m1 * BL],
                        in_=ps[:], func=AF.Tanh)
                if t == CS // 2 and mid_cb is not None:
                    mid_cb(arch)
                if t >= 6:
                    budget = 6
                    while items and budget > 0:
                        items.pop(0)()
                        budget -= 1
            for it in items:
                it()
            return arch

        # ---- Prologue: slabs + pre(0), pre(1) via the local-slab path ----
        # (wi_sb is [Wi1-tiles] on layer-1 cores, zeros on layer-2 cores, so
        # layer-2 warm-up chunks get pre = 0.)
        pres = {}
        pro_slabs = []
        for pi in range(LAG):
            xes = emit_gathers(pi)
            slab = slpool.tile([P, CW], CDT, tag="slab", name=f"slab{pi}")
            for it in slab_items(pi, slab, xes, zero_tail=True):
                it()
            pro_slabs.append(slab)
        pre0 = prepool.tile([P, CW], CDT, tag="pre", name="pre0")
        for half in range(2):
            for it in prod_items("w0", pro_slabs[0], pre0, wi_sb, KE, half):
                it()
        pres[0] = pre0

        # ---- Main pipelined loop ----
        arch_prev = None
        combs = {}
        for c in range(NCH + LAG):
            items = []
            slab = None
            # embedding slab for token chunk c+LAG (clamped; junk past the end)
            if c <= NCH - 1:
                tc_ = min(c + LAG, NCH - 1)
                xes = emit_gathers(tc_)
                slab = slpool.tile([P, CW], CDT, tag="slab", name=f"slabm{c}")
                items += slab_items(f"m{c}", slab, xes, zero_tail=False)
            # pre(1) production deferred into chunk 0's tanh windows
            if c == 0:
                pre1t = prepool.tile([P, CW], CDT, tag="pre", name="pre1")
                for half in range(2):
                    items += prod_items("w1", pro_slabs[1], pre1t, wi_sb,
                                        KE, half)
                pres[1] = pre1t
            # production of pre(c+1) from comb(c-1), per half-chunk so the
            # first half can start as soon as its SRC gather lands
            if 1 <= c <= NCH:
                comb_p = combs.pop(c - 1)
                src = srpool.tile([P, CW], CDT, tag="src", name=f"src{c}")
                pre_n = prepool.tile([P, CW], CDT, tag="pre", name=f"pre{c + 1}")
                for half in range(2):
                    nc.gpsimd.indirect_dma_start(
                        out=src[:, half * HCW:(half + 1) * HCW],
                        out_offset=None, in_=comb_p[half][:],
                        in_offset=IndirectOffsetOnAxis(ap=sel_sb[:, 0:1],
                                                       axis=0))
                    items += prod_items(f"p{c}", src, pre_n, wprod_sb, KH, half)
                pres[c + 1] = pre_n

            mid_cb = None
            if c <= NCH - 1:
                sends = [cpool.tile([P, HCW], CDT, space="DRAM",
                                    name=f"send_db{c}_{h}") for h in range(2)]
                comb = [cpool.tile([3 * P, HCW], CDT, space="DRAM",
                                   name=f"comb{c}_{h}") for h in range(2)]

                def halfio(arch, half, sends=sends, comb=comb, slab=slab):
                    h0, h1 = half * HCW, (half + 1) * HCW
                    nc.sync.dma_start(out=sends[half][:], in_=arch[:, h0:h1])
                    if local_cc:
                        nc.gpsimd.dma_start(out=comb[half][0:P, :],
                                            in_=sends[half][:])
                    else:
                        nc.gpsimd.collective_compute(
                            "AllGather", mybir.AluOpType.bypass,
                            ins=[sends[half][:]], outs=[comb[half][0:2 * P, :]],
                            replica_groups=REPLICA_GROUPS)
                    nc.sync.dma_start(out=comb[half][2 * P:3 * P, :],
                                      in_=slab[:, h0:h1])

                mid_cb = lambda arch, f=halfio: f(arch, 0)
                combs[c] = comb

            arch_prev = scan_chunk(c, pres.pop(c), arch_prev, items, mid_cb)

            if c <= NCH - 1:
                halfio(arch_prev, 1)

        # ---- head ----
        if True:
            hps = spApool.tile([BL, 1], F32, tag="sp", name="hps")
            for k in range(KH):
                last = arch_prev[:, (CS - 1) * W + k * BL:
                                 (CS - 1) * W + (k + 1) * BL]
                nc.tensor.matmul(
                    hps[:], lhsT=last,
                    rhs=wd_sb[:, k:k + 1], start=(k == 0), stop=(k == KH - 1))
            y_sb = const_pool.tile([P, 1], F32, name="y_sb")
            nc.scalar.activation(out=y_sb[0:BL, 0:1], in_=hps[:],
                                 func=AF.Sigmoid, bias=bd_sb[0:BL, 0:1])
            nc.sync.dma_start(out=y_d[:], in_=y_sb[0:BL, 0:1])

    nc.compile()
    return nc


def _prep_maps(x, emb, Wi1, Wh1, b1, Wi2, Wh2, b2, Wd, bd):
    f = NP_CDT
    x = np.asarray(x, np.int32)
    shared = {
        "emb": np.ascontiguousarray(np.asarray(emb, f)),
        "wdk": np.ascontiguousarray(np.asarray(Wd, f).reshape(KH, P).T),
        "bdv": np.ascontiguousarray(np.broadcast_to(
            np.asarray(bd, np.float32), (BL,))),
    }
    wi1_f = np.asarray(Wi1, f)
    wprod_a = np.concatenate([wi1_f, np.zeros((HID - EMB, HID), f)])
    wprod_b = np.ascontiguousarray(np.asarray(Wi2, f))
    wi1_b = np.zeros((EMB, HID), f)
    b_a = np.ascontiguousarray(np.asarray(b1, np.float32).reshape(M, P))
    b_b = np.ascontiguousarray(np.asarray(b2, np.float32).reshape(M, P))
    wh1 = np.ascontiguousarray(np.asarray(Wh1, f))
    wh2 = np.ascontiguousarray(np.asarray(Wh2, f))
    sel_a = (2 * P + np.arange(P, dtype=np.int32)).reshape(P, 1)
    sel_b = np.arange(P, dtype=np.int32).reshape(P, 1)
    in_maps = []
    for c in range(NCORES):
        p = c % NPAIR
        xs = x[p * BL:(p + 1) * BL, :]                    # [16, 512]
        xgrp = np.ascontiguousarray(xs.T).reshape(-1, P)  # (t, b) order
        role_a = c < NPAIR
        in_maps.append({
            **shared,
            "xgt": np.ascontiguousarray(xgrp.T),          # [P, SBL]
            "whs": wh1 if role_a else wh2,
            "wprod": wprod_a if role_a else wprod_b,
            "wi1": np.ascontiguousarray(wi1_f) if role_a else wi1_b,
            "bprod": b_a if role_a else b_b,
            "sel": sel_a if role_a else sel_b,
        })
    return in_maps


def kernel(x, emb, Wi1, Wh1, b1, Wi2, Wh2, b2, Wd, bd):
    if "nc" not in _BUILT:
        _BUILT["nc"] = build()
    nc = _BUILT["nc"]
    in_maps = _prep_maps(x, emb, Wi1, Wh1, b1, Wi2, Wh2, b2, Wd, bd)
    res = run_bass_kernel_spmd(nc, in_maps, list(range(NCORES)))
    kernel.last_result = res
    y = np.concatenate([np.asarray(res.results[NPAIR + p]["y"], np.float32)
                        for p in range(NPAIR)])
    return y
